# revision 21
# baseline (speedup 1.0000x reference)
"""Trainium2 Bass kernel for MemoryBankNet loss (scatter_memory).

Computes, for inputs/backbone_inputs [256,512], targets [256], memory_features
[100000,512]:
    ce   = cross_entropy(l2norm(inputs) @ mem.T / 0.05, targets)
    dist = (0.007/0.3) * ||l2norm(backbone_inputs) - mem[targets[j//4]]||_F
    out  = ce + dist                                    (f32 scalar)

Distribution: classes (mem rows) are sharded 12500/core across 8 NeuronCores
(tensor parallel over the class axis).  Each core computes its partial softmax
denominator with a fixed log-shift; the tiny [256] partials are combined on
host (the "all-reduce" of the softmax normalizer).  The B-row side terms
(target logits for the CE numerator, distill partials) are exact host numpy
over [256,512] gathers -- the same host routing of target rows the hint
describes.

Device numerics: the memory bank and the (pre-normalized, pre-temperature-
scaled) inputs are quantized host-side to fp8e4m3 with power-of-2 scales
(mem*32, l2norm(inp)*20*32), so PSUM = 1024*logit and the activation applies
a CONSTANT scale 2^-10 and bias -104 -- no per-row scale tensors on device.
Validated host-side: rel err ~5.5e-4 vs f64 (tolerance 2e-2).

Device layout per core:
  memT [128, 24*2048+1024] fp8: shard packed as [p][substrip j][k2][i][c]
    with d = k2*256 + i*128 + p, c = class within substrip (512 per substrip,
    tail substrip 256 incl. 44 zero-padded classes -> exp underflows to 0).
  matmul: DoubleRow fp8 (2 fp8/cell, 256-deep contraction per pass):
    stationary = inpT [128, 2, 128] (one b-half, one k2), moving = mem
    substrip [128, 2, 512], psum [128b, 512c], accumulated over 2 k2 passes.
  Per round (2 substrips): 8 MMs fill 4 psum banks; one ACT Exp drains all 4
  into bf16 scratch; DVE row-reduces per half into the sumexp accumulator.
"""

import numpy as np
import ml_dtypes

import concourse.bass as bass
import concourse.tile as tile
from concourse import bacc, mybir
from concourse.bass_utils import run_bass_kernel_spmd

F32 = mybir.dt.float32
FP8 = mybir.dt.float8e4
BF16 = mybir.dt.bfloat16
AF = mybir.ActivationFunctionType
AX = mybir.AxisListType
DR = mybir.MatmulPerfMode.DoubleRow

N_CORES = 8
B, D, C = 256, 512, 100000
CT = 512                     # classes per substrip (one psum bank)
NSUB = 24                    # substrips per core
CS = NSUB * CT               # 12288 device classes per core
CHOST = C - N_CORES * CS     # 1696 remainder classes summed exactly on host
SUB_B = 2 * 2 * CT           # 2048 bytes/partition per substrip
TOT_B = NSUB * SUB_B

TEMP = 0.05
SHIFT = 104.0                # fixed log-shift
ASCALE = 2.0 ** -10          # undo fp8 scales 32*32 = 1024
DISTILL_SCALE = 0.007 / 0.3
EPS = 1e-12

_PROGRAM = None
_last_in_maps = None


def _build_program():
    nc = bacc.Bacc("TRN2", target_bir_lowering=False, debug=False,
                   num_devices=N_CORES)
    memT = nc.dram_tensor("memT", [128, TOT_B], FP8, kind="ExternalInput").ap()
    # stationary: [p][h][k2][i][m] fp8, d = k2*256 + i*128 + p, row = h*128+m
    inpT = nc.dram_tensor("inpT", [128, 2, 2, 2, 128], FP8,
                          kind="ExternalInput").ap()
    # per-(h, group) softmax partials; host does the final 6-column sum
    out = nc.dram_tensor("out", [128, 12], F32, kind="ExternalOutput").ap()

    with tile.TileContext(nc) as tc:
        _body(tc, nc, memT, inpT, out)

    nc.compile()
    return nc


def _body(tc, nc, memT, inpT, out):
    NG = 6                   # ACT groups of 4 substrips = 1MB DMA strips
    NWARM = 5                # junk MMs bridge preamble-end -> strip0 arrival

    with (
        tc.tile_pool(name="const", bufs=1) as cpool,
        tc.tile_pool(name="exps", bufs=3) as epool,
        tc.tile_pool(name="psum", bufs=2, space="PSUM") as ppool,
    ):
        wt = cpool.tile([128, 2, 2, 2, 128], FP8, tag="wt", name="wt")
        saccw = cpool.tile([128, 2 * NG], F32, tag="saccw", name="saccw")
        nbias = cpool.tile([128, 1], F32, tag="nbias", name="nbias")
        jm = cpool.tile([128, 1024], FP8, tag="jm", name="jm")
        # whole 6.3MB shard is SBUF-resident: strips are persistent tiles,
        # every DMA trigger issues at t=0 with no buffer-reuse waits.
        # All strips ride the sync HWDGE ring alone (uncontended it runs
        # ~340 GB/s; a second queue halves both).  Trigger-issuer choice
        # matters: the 3rd outstanding DMA on a ring BLOCKS its issuing
        # engine, so only the sync engine (otherwise idle) may carry the
        # strip queue; the ACT ring gets just the tiny stationary.
        strips = [cpool.tile([128, 4, 2, 2, CT], FP8, tag=f"mt{s}",
                             name=f"mt{s}") for s in range(NG)]
        nc.scalar.dma_start(wt[:], inpT)
        # strip 0 goes as two 512KB halves so the first matmuls fire ~2us
        # sooner; the rest as full 1MB strips (~400 GB/s on this ring)
        nc.sync.dma_start(
            strips[0][:, 0:2].rearrange("p w k i c -> p (w k i c)"),
            memT[:, 0:2 * SUB_B])
        nc.sync.dma_start(
            strips[0][:, 2:4].rearrange("p w k i c -> p (w k i c)"),
            memT[:, 2 * SUB_B:4 * SUB_B])
        for s in range(1, NG):
            nc.sync.dma_start(
                strips[s][:].rearrange("p w k i c -> p (w k i c)"),
                memT[:, s * 4 * SUB_B:(s + 1) * 4 * SUB_B])
        nc.vector.memset(nbias[:], -SHIFT)
        nc.vector.memset(jm[:], 0.0)

        # warm-up: zero matmuls keep the PE busy from preamble-end until
        # strip 0 lands, so the HAM clock-gate hits 8/8 as real work starts
        ps0 = ppool.tile([128, 4, CT], F32, tag="ps", name="ps")
        jw = jm[:, 0:256].rearrange("p (i m) -> p i m", i=2)
        jr = jm[:].rearrange("p (i c) -> p i c", i=2)
        for _ in range(NWARM):
            nc.tensor.matmul(ps0[:, 0, :], jw, jr, start=True, stop=True,
                             perf_mode=DR, skip_group_check=True)

        # Each group: per half h, 8 matmuls fill 4 psum banks, then a single
        # ACT Exp drains them, accum_out producing the (h, group) softmax
        # partial.  PE fills the other half's banks while ACT drains --
        # psum pool holds 2x4 banks.
        for g in range(NG):
            for h in range(2):
                ps = ps0 if (g == 0 and h == 0) else ppool.tile(
                    [128, 4, CT], F32, tag="ps", name="ps")
                for k2 in range(2):
                    for jj in range(4):
                        nc.tensor.matmul(
                            ps[:, jj, :],
                            wt[:, h, k2],
                            strips[g][:, jj, k2],
                            start=(k2 == 0), stop=(k2 == 1),
                            perf_mode=DR, skip_group_check=True)

                ex = epool.tile([128, 4 * CT], BF16, tag="ex", name="ex")
                col = saccw[:, h * NG + g:h * NG + g + 1]
                # h=0 rounds (and the final round, which is on the critical
                # tail) fold the row-sum into the ACT accumulator; other h=1
                # rounds leave it to the otherwise-idle DVE, saving the
                # 187ns accumulator read on those ACT instructions
                on_act = h == 0 or g == NG - 1
                nc.scalar.activation(
                    ex[:].rearrange("p (b c) -> p b c", c=CT),
                    ps[:],
                    AF.Exp, bias=nbias[:], scale=ASCALE,
                    accum_out=col if on_act else None)
                if not on_act:
                    nc.vector.reduce_sum(col, ex[:], axis=AX.X)

        nc.sync.dma_start(out, saccw[:])


def _get_program():
    global _PROGRAM
    if _PROGRAM is None:
        _PROGRAM = _build_program()
    return _PROGRAM


def _quant_fp8(x):
    return np.clip(x, -240.0, 240.0).astype(ml_dtypes.float8_e4m3)


def kernel(backbone_inputs, inputs, targets, memory_features, **_unused):
    x = np.ascontiguousarray(inputs, dtype=np.float32)
    bb = np.ascontiguousarray(backbone_inputs, dtype=np.float32)
    mem = np.ascontiguousarray(memory_features, dtype=np.float32)
    tgt = np.asarray(targets).astype(np.int64)

    xn = x / np.maximum(np.linalg.norm(x, axis=1, keepdims=True), EPS)

    # stationary: value[p,h,k2,i,m] = (xn*640)[h*128+m, k2*256+i*128+p]
    qi = _quant_fp8(xn * (32.0 / TEMP))
    spt8 = np.ascontiguousarray(
        qi.reshape(2, 128, 2, 2, 128).transpose(4, 0, 2, 3, 1))

    qm = _quant_fp8(mem * 32.0)

    nc = _get_program()
    in_maps = []
    for c in range(N_CORES):
        sh = qm[c * CS:(c + 1) * CS].reshape(NSUB, CT, 2, 2, 128)
        shard = np.ascontiguousarray(sh.transpose(4, 0, 2, 3, 1)).reshape(128, -1)
        in_maps.append({"memT": shard, "inpT": spt8})
    global _last_in_maps
    _last_in_maps = in_maps
    results = run_bass_kernel_spmd(nc, in_maps, core_ids=list(range(N_CORES)))

    s_tot = np.zeros(B, dtype=np.float64)
    for r in results.results:
        o = r["out"].astype(np.float64)        # [128, 2*6] (h, group) partials
        s_tot += np.concatenate([o[:, 0:6].sum(axis=1), o[:, 6:12].sum(axis=1)])
    # remainder classes (C - 8*CS = 1696): exact on host
    lt = (xn @ mem[N_CORES * CS:].T.astype(np.float64)) / TEMP
    s_tot += np.exp(lt - SHIFT).sum(axis=1)

    # host: exact B-row side terms (target-row routing per sharding hint)
    lse = SHIFT + np.log(s_tot)
    tl = np.einsum("bd,bd->b", xn, mem[tgt], dtype=np.float64) / TEMP
    ce = float(np.mean(lse - tl))
    bbn = bb / np.maximum(np.linalg.norm(bb, axis=1, keepdims=True), EPS)
    g2 = mem[tgt[np.arange(B) // 4]]
    dist = DISTILL_SCALE * float(
        np.sqrt(((bbn.astype(np.float64) - g2) ** 2).sum()))
    return np.asarray(ce + dist, dtype=np.float32)


# revision 23
# speedup vs baseline: 1.0467x; 1.0467x over previous
"""Trainium2 Bass kernel for MemoryBankNet loss (scatter_memory).

Computes, for inputs/backbone_inputs [256,512], targets [256], memory_features
[100000,512]:
    ce   = cross_entropy(l2norm(inputs) @ mem.T / 0.05, targets)
    dist = (0.007/0.3) * ||l2norm(backbone_inputs) - mem[targets[j//4]]||_F
    out  = ce + dist                                    (f32 scalar)

Distribution: classes (mem rows) are sharded 12500/core across 8 NeuronCores
(tensor parallel over the class axis).  Each core computes its partial softmax
denominator with a fixed log-shift; the tiny [256] partials are combined on
host (the "all-reduce" of the softmax normalizer).  The B-row side terms
(target logits for the CE numerator, distill partials) are exact host numpy
over [256,512] gathers -- the same host routing of target rows the hint
describes.

Device numerics: the memory bank and the (pre-normalized, pre-temperature-
scaled) inputs are quantized host-side to fp8e4m3 with power-of-2 scales
(mem*32, l2norm(inp)*20*32), so PSUM = 1024*logit and the activation applies
a CONSTANT scale 2^-10 and bias -104 -- no per-row scale tensors on device.
Validated host-side: rel err ~5.5e-4 vs f64 (tolerance 2e-2).

Device layout per core:
  memT [128, 24*2048+1024] fp8: shard packed as [p][substrip j][k2][i][c]
    with d = k2*256 + i*128 + p, c = class within substrip (512 per substrip,
    tail substrip 256 incl. 44 zero-padded classes -> exp underflows to 0).
  matmul: DoubleRow fp8 (2 fp8/cell, 256-deep contraction per pass):
    stationary = inpT [128, 2, 128] (one b-half, one k2), moving = mem
    substrip [128, 2, 512], psum [128b, 512c], accumulated over 2 k2 passes.
  Per round (2 substrips): 8 MMs fill 4 psum banks; one ACT Exp drains all 4
  into bf16 scratch; DVE row-reduces per half into the sumexp accumulator.
"""

import numpy as np
import ml_dtypes

import concourse.bass as bass
import concourse.tile as tile
from concourse import bacc, mybir
from concourse.bass_utils import run_bass_kernel_spmd

F32 = mybir.dt.float32
FP8 = mybir.dt.float8e4
BF16 = mybir.dt.bfloat16
AF = mybir.ActivationFunctionType
AX = mybir.AxisListType
DR = mybir.MatmulPerfMode.DoubleRow

N_CORES = 8
B, D, C = 256, 512, 100000
CT = 512                     # classes per substrip (one psum bank)
NSUB = 24                    # substrips per core
CS = NSUB * CT               # 12288 device classes per core
CHOST = C - N_CORES * CS     # 1696 remainder classes summed exactly on host
SUB_B = 2 * 2 * CT           # 2048 bytes/partition per substrip
TOT_B = NSUB * SUB_B

TEMP = 0.05
SHIFT = 104.0                # fixed log-shift
ASCALE = 2.0 ** -10          # undo fp8 scales 32*32 = 1024
DISTILL_SCALE = 0.007 / 0.3
EPS = 1e-12

_PROGRAM = None
_last_in_maps = None


def _build_program():
    nc = bacc.Bacc("TRN2", target_bir_lowering=False, debug=False,
                   num_devices=N_CORES)
    memT = nc.dram_tensor("memT", [128, TOT_B], FP8, kind="ExternalInput").ap()
    # stationary: [p][h][k2][i][m] fp8, d = k2*256 + i*128 + p, row = h*128+m
    inpT = nc.dram_tensor("inpT", [128, 2, 2, 2, 128], FP8,
                          kind="ExternalInput").ap()
    # per-(h, group) softmax partials; host does the final 6-column sum
    out = nc.dram_tensor("out", [128, 12], F32, kind="ExternalOutput").ap()

    with tile.TileContext(nc) as tc:
        _body(tc, nc, memT, inpT, out)

    nc.compile()
    return nc


def _body(tc, nc, memT, inpT, out):
    NG = 6                   # ACT groups of 4 substrips = 1MB DMA strips
    NWARM = 9                # junk MMs span a full ~3.4us HAM window at the
                             # cold 427ns issue cadence, ending ~strip0-ready

    with (
        tc.tile_pool(name="const", bufs=1) as cpool,
        tc.tile_pool(name="exps", bufs=3) as epool,
        tc.tile_pool(name="psum", bufs=2, space="PSUM") as ppool,
    ):
        wt = cpool.tile([128, 2, 2, 2, 128], FP8, tag="wt", name="wt")
        saccw = cpool.tile([128, 2 * NG], F32, tag="saccw", name="saccw")
        nbias = cpool.tile([128, 1], F32, tag="nbias", name="nbias")
        jm = cpool.tile([128, 1024], FP8, tag="jm", name="jm")
        # whole 6.3MB shard is SBUF-resident: strips are persistent tiles,
        # every DMA trigger issues at t=0 with no buffer-reuse waits.
        # All strips ride the sync HWDGE ring alone (uncontended it runs
        # ~340 GB/s; a second queue halves both).  Trigger-issuer choice
        # matters: the 3rd outstanding DMA on a ring BLOCKS its issuing
        # engine, so only the sync engine (otherwise idle) may carry the
        # strip queue; the ACT ring gets just the tiny stationary.
        strips = [cpool.tile([128, 4, 2, 2, CT], FP8, tag=f"mt{s}",
                             name=f"mt{s}") for s in range(NG)]
        nc.scalar.dma_start(wt[:], inpT)
        # strip 0 goes as two 512KB halves so the first matmuls fire ~2us
        # sooner; the rest as full 1MB strips (~400 GB/s on this ring)
        nc.sync.dma_start(
            strips[0][:, 0:2].rearrange("p w k i c -> p (w k i c)"),
            memT[:, 0:2 * SUB_B])
        nc.sync.dma_start(
            strips[0][:, 2:4].rearrange("p w k i c -> p (w k i c)"),
            memT[:, 2 * SUB_B:4 * SUB_B])
        for s in range(1, NG):
            nc.sync.dma_start(
                strips[s][:].rearrange("p w k i c -> p (w k i c)"),
                memT[:, s * 4 * SUB_B:(s + 1) * 4 * SUB_B])
        nc.vector.memset(nbias[:], -SHIFT)
        nc.vector.memset(jm[:], 0.0)

        # warm-up: zero matmuls keep the PE busy from preamble-end until
        # strip 0 lands, so the HAM clock-gate hits 8/8 as real work starts
        ps0 = ppool.tile([128, 4, CT], F32, tag="ps", name="ps")
        jw = jm[:, 0:256].rearrange("p (i m) -> p i m", i=2)
        jr = jm[:].rearrange("p (i c) -> p i c", i=2)
        for _ in range(NWARM):
            nc.tensor.matmul(ps0[:, 0, :], jw, jr, start=True, stop=True,
                             perf_mode=DR, skip_group_check=True)

        # Each group: per half h, 8 matmuls fill 4 psum banks, then a single
        # ACT Exp drains them, accum_out producing the (h, group) softmax
        # partial.  PE fills the other half's banks while ACT drains --
        # psum pool holds 2x4 banks.
        for g in range(NG):
            for h in range(2):
                ps = ps0 if (g == 0 and h == 0) else ppool.tile(
                    [128, 4, CT], F32, tag="ps", name="ps")
                for k2 in range(2):
                    for jj in range(4):
                        nc.tensor.matmul(
                            ps[:, jj, :],
                            wt[:, h, k2],
                            strips[g][:, jj, k2],
                            start=(k2 == 0), stop=(k2 == 1),
                            perf_mode=DR, skip_group_check=True)

                ex = epool.tile([128, 4 * CT], BF16, tag="ex", name="ex")
                col = saccw[:, h * NG + g:h * NG + g + 1]
                # h=0 rounds (and the final round, which is on the critical
                # tail) fold the row-sum into the ACT accumulator; other h=1
                # rounds leave it to the otherwise-idle DVE, saving the
                # 187ns accumulator read on those ACT instructions
                on_act = h == 0 or g == NG - 1
                nc.scalar.activation(
                    ex[:],
                    ps[:].rearrange("p b c -> p (b c)"),
                    AF.Exp, bias=nbias[:], scale=ASCALE,
                    accum_out=col if on_act else None)
                if not on_act:
                    nc.vector.reduce_sum(col, ex[:], axis=AX.X)

        nc.sync.dma_start(out, saccw[:])


def _get_program():
    global _PROGRAM
    if _PROGRAM is None:
        _PROGRAM = _build_program()
    return _PROGRAM


def _quant_fp8(x):
    return np.clip(x, -240.0, 240.0).astype(ml_dtypes.float8_e4m3)


def kernel(backbone_inputs, inputs, targets, memory_features, **_unused):
    x = np.ascontiguousarray(inputs, dtype=np.float32)
    bb = np.ascontiguousarray(backbone_inputs, dtype=np.float32)
    mem = np.ascontiguousarray(memory_features, dtype=np.float32)
    tgt = np.asarray(targets).astype(np.int64)

    xn = x / np.maximum(np.linalg.norm(x, axis=1, keepdims=True), EPS)

    # stationary: value[p,h,k2,i,m] = (xn*640)[h*128+m, k2*256+i*128+p]
    qi = _quant_fp8(xn * (32.0 / TEMP))
    spt8 = np.ascontiguousarray(
        qi.reshape(2, 128, 2, 2, 128).transpose(4, 0, 2, 3, 1))

    qm = _quant_fp8(mem * 32.0)

    nc = _get_program()
    in_maps = []
    for c in range(N_CORES):
        sh = qm[c * CS:(c + 1) * CS].reshape(NSUB, CT, 2, 2, 128)
        shard = np.ascontiguousarray(sh.transpose(4, 0, 2, 3, 1)).reshape(128, -1)
        in_maps.append({"memT": shard, "inpT": spt8})
    global _last_in_maps
    _last_in_maps = in_maps
    results = run_bass_kernel_spmd(nc, in_maps, core_ids=list(range(N_CORES)))

    s_tot = np.zeros(B, dtype=np.float64)
    for r in results.results:
        o = r["out"].astype(np.float64)        # [128, 2*6] (h, group) partials
        s_tot += np.concatenate([o[:, 0:6].sum(axis=1), o[:, 6:12].sum(axis=1)])
    # remainder classes (C - 8*CS = 1696): exact on host
    lt = (xn @ mem[N_CORES * CS:].T.astype(np.float64)) / TEMP
    s_tot += np.exp(lt - SHIFT).sum(axis=1)

    # host: exact B-row side terms (target-row routing per sharding hint)
    lse = SHIFT + np.log(s_tot)
    tl = np.einsum("bd,bd->b", xn, mem[tgt], dtype=np.float64) / TEMP
    ce = float(np.mean(lse - tl))
    bbn = bb / np.maximum(np.linalg.norm(bb, axis=1, keepdims=True), EPS)
    g2 = mem[tgt[np.arange(B) // 4]]
    dist = DISTILL_SCALE * float(
        np.sqrt(((bbn.astype(np.float64) - g2) ** 2).sum()))
    return np.asarray(ce + dist, dtype=np.float32)


# revision 24
# speedup vs baseline: 1.0824x; 1.0342x over previous
"""Trainium2 Bass kernel for MemoryBankNet loss (scatter_memory).

Computes, for inputs/backbone_inputs [256,512], targets [256], memory_features
[100000,512]:
    ce   = cross_entropy(l2norm(inputs) @ mem.T / 0.05, targets)
    dist = (0.007/0.3) * ||l2norm(backbone_inputs) - mem[targets[j//4]]||_F
    out  = ce + dist                                    (f32 scalar)

Distribution: classes (mem rows) are sharded 12500/core across 8 NeuronCores
(tensor parallel over the class axis).  Each core computes its partial softmax
denominator with a fixed log-shift; the tiny [256] partials are combined on
host (the "all-reduce" of the softmax normalizer).  The B-row side terms
(target logits for the CE numerator, distill partials) are exact host numpy
over [256,512] gathers -- the same host routing of target rows the hint
describes.

Device numerics: the memory bank and the (pre-normalized, pre-temperature-
scaled) inputs are quantized host-side to fp8e4m3 with power-of-2 scales
(mem*32, l2norm(inp)*20*32), so PSUM = 1024*logit and the activation applies
a CONSTANT scale 2^-10 and bias -104 -- no per-row scale tensors on device.
Validated host-side: rel err ~5.5e-4 vs f64 (tolerance 2e-2).

Device layout per core:
  memT [128, 24*2048+1024] fp8: shard packed as [p][substrip j][k2][i][c]
    with d = k2*256 + i*128 + p, c = class within substrip (512 per substrip,
    tail substrip 256 incl. 44 zero-padded classes -> exp underflows to 0).
  matmul: DoubleRow fp8 (2 fp8/cell, 256-deep contraction per pass):
    stationary = inpT [128, 2, 128] (one b-half, one k2), moving = mem
    substrip [128, 2, 512], psum [128b, 512c], accumulated over 2 k2 passes.
  Per round (2 substrips): 8 MMs fill 4 psum banks; one ACT Exp drains all 4
  into bf16 scratch; DVE row-reduces per half into the sumexp accumulator.
"""

import numpy as np
import ml_dtypes

import concourse.bass as bass
import concourse.tile as tile
from concourse import bacc, mybir
from concourse.bass_utils import run_bass_kernel_spmd

F32 = mybir.dt.float32
FP8 = mybir.dt.float8e4
BF16 = mybir.dt.bfloat16
AF = mybir.ActivationFunctionType
AX = mybir.AxisListType
DR = mybir.MatmulPerfMode.DoubleRow

N_CORES = 8
B, D, C = 256, 512, 100000
CT = 512                     # classes per substrip (one psum bank)
NSUB = 24                    # substrips per core
CS = NSUB * CT               # 12288 device classes per core
CHOST = C - N_CORES * CS     # 1696 remainder classes summed exactly on host
SUB_B = 2 * 2 * CT           # 2048 bytes/partition per substrip
TOT_B = NSUB * SUB_B

TEMP = 0.05
SHIFT = 104.0                # fixed log-shift
ASCALE = 2.0 ** -10          # undo fp8 scales 32*32 = 1024
DISTILL_SCALE = 0.007 / 0.3
EPS = 1e-12

_PROGRAM = None
_last_in_maps = None


def _build_program():
    nc = bacc.Bacc("TRN2", target_bir_lowering=False, debug=False,
                   num_devices=N_CORES)
    memT = nc.dram_tensor("memT", [128, TOT_B], FP8, kind="ExternalInput").ap()
    # stationary: [p][h][k2][i][m] fp8, d = k2*256 + i*128 + p, row = h*128+m
    inpT = nc.dram_tensor("inpT", [128, 2, 2, 2, 128], FP8,
                          kind="ExternalInput").ap()
    # per-(h, group) softmax partials; host does the final 6-column sum
    out = nc.dram_tensor("out", [128, 12], F32, kind="ExternalOutput").ap()

    with tile.TileContext(nc) as tc:
        _body(tc, nc, memT, inpT, out)

    nc.compile()
    return nc


def _body(tc, nc, memT, inpT, out):
    NG = 6                   # ACT groups of 4 substrips = 1MB DMA strips
    NWARM = 9                # junk MMs span a full ~3.4us HAM window at the
                             # cold 427ns issue cadence, ending ~strip0-ready

    with (
        tc.tile_pool(name="const", bufs=1) as cpool,
        tc.tile_pool(name="exps", bufs=3) as epool,
        tc.tile_pool(name="psum", bufs=2, space="PSUM") as ppool,
    ):
        wt = cpool.tile([128, 2, 2, 2, 128], FP8, tag="wt", name="wt")
        saccw = cpool.tile([128, 2 * NG], F32, tag="saccw", name="saccw")
        nbias = cpool.tile([128, 1], F32, tag="nbias", name="nbias")
        jm = cpool.tile([128, 1024], FP8, tag="jm", name="jm")
        # whole 6.3MB shard is SBUF-resident: strips are persistent tiles,
        # every DMA trigger issues at t=0 with no buffer-reuse waits.
        # All strips ride the sync HWDGE ring alone (uncontended it runs
        # ~340 GB/s; a second queue halves both).  Trigger-issuer choice
        # matters: the 3rd outstanding DMA on a ring BLOCKS its issuing
        # engine, so only the sync engine (otherwise idle) may carry the
        # strip queue; the ACT ring gets just the tiny stationary.
        strips = [cpool.tile([128, 4, 2, 2, CT], FP8, tag=f"mt{s}",
                             name=f"mt{s}") for s in range(NG)]
        nc.scalar.dma_start(wt[:], inpT)
        # strip 0 goes as two 512KB halves so the first matmuls fire ~2us
        # sooner; the rest as full 1MB strips (~400 GB/s on this ring)
        nc.sync.dma_start(
            strips[0][:, 0:2].rearrange("p w k i c -> p (w k i c)"),
            memT[:, 0:2 * SUB_B])
        nc.sync.dma_start(
            strips[0][:, 2:4].rearrange("p w k i c -> p (w k i c)"),
            memT[:, 2 * SUB_B:4 * SUB_B])
        for s in range(1, NG):
            nc.sync.dma_start(
                strips[s][:].rearrange("p w k i c -> p (w k i c)"),
                memT[:, s * 4 * SUB_B:(s + 1) * 4 * SUB_B])
        nc.vector.memset(nbias[:], -SHIFT)
        nc.vector.memset(jm[:], 0.0)

        # warm-up: zero matmuls keep the PE busy from preamble-end until
        # strip 0 lands, so the HAM clock-gate hits 8/8 as real work starts
        ps0 = ppool.tile([128, 4, CT], F32, tag="ps", name="ps")
        jw = jm[:, 0:256].rearrange("p (i m) -> p i m", i=2)
        jr = jm[:].rearrange("p (i c) -> p i c", i=2)
        for _ in range(NWARM):
            nc.tensor.matmul(ps0[:, 0, :], jw, jr, start=True, stop=True,
                             perf_mode=DR, skip_group_check=True)

        # Each group: per half h, 8 matmuls fill 4 psum banks, then a single
        # ACT Exp drains them, accum_out producing the (h, group) softmax
        # partial.  PE fills the other half's banks while ACT drains --
        # psum pool holds 2x4 banks.
        for g in range(NG):
            for h in range(2):
                ps = ps0 if (g == 0 and h == 0) else ppool.tile(
                    [128, 4, CT], F32, tag="ps", name="ps")
                for k2 in range(2):
                    for jj in range(4):
                        nc.tensor.matmul(
                            ps[:, jj, :],
                            wt[:, h, k2],
                            strips[g][:, jj, k2],
                            start=(k2 == 0), stop=(k2 == 1),
                            perf_mode=DR, skip_group_check=True)

                ex = epool.tile([128, 4 * CT], BF16, tag="ex", name="ex")
                col = saccw[:, h * NG + g:h * NG + g + 1]
                # only the final group's rounds (on the critical out-DMA
                # tail) fold the row-sum into the ACT accumulator; all other
                # rounds leave it to the otherwise-idle DVE, saving the
                # 187ns accumulator read on those ACT instructions
                on_act = g == NG - 1
                nc.scalar.activation(
                    ex[:],
                    ps[:].rearrange("p b c -> p (b c)"),
                    AF.Exp, bias=nbias[:], scale=ASCALE,
                    accum_out=col if on_act else None)
                if not on_act:
                    nc.vector.reduce_sum(col, ex[:], axis=AX.X)

        nc.sync.dma_start(out, saccw[:])


def _get_program():
    global _PROGRAM
    if _PROGRAM is None:
        _PROGRAM = _build_program()
    return _PROGRAM


def _quant_fp8(x):
    return np.clip(x, -240.0, 240.0).astype(ml_dtypes.float8_e4m3)


def kernel(backbone_inputs, inputs, targets, memory_features, **_unused):
    x = np.ascontiguousarray(inputs, dtype=np.float32)
    bb = np.ascontiguousarray(backbone_inputs, dtype=np.float32)
    mem = np.ascontiguousarray(memory_features, dtype=np.float32)
    tgt = np.asarray(targets).astype(np.int64)

    xn = x / np.maximum(np.linalg.norm(x, axis=1, keepdims=True), EPS)

    # stationary: value[p,h,k2,i,m] = (xn*640)[h*128+m, k2*256+i*128+p]
    qi = _quant_fp8(xn * (32.0 / TEMP))
    spt8 = np.ascontiguousarray(
        qi.reshape(2, 128, 2, 2, 128).transpose(4, 0, 2, 3, 1))

    qm = _quant_fp8(mem * 32.0)

    nc = _get_program()
    in_maps = []
    for c in range(N_CORES):
        sh = qm[c * CS:(c + 1) * CS].reshape(NSUB, CT, 2, 2, 128)
        shard = np.ascontiguousarray(sh.transpose(4, 0, 2, 3, 1)).reshape(128, -1)
        in_maps.append({"memT": shard, "inpT": spt8})
    global _last_in_maps
    _last_in_maps = in_maps
    results = run_bass_kernel_spmd(nc, in_maps, core_ids=list(range(N_CORES)))

    s_tot = np.zeros(B, dtype=np.float64)
    for r in results.results:
        o = r["out"].astype(np.float64)        # [128, 2*6] (h, group) partials
        s_tot += np.concatenate([o[:, 0:6].sum(axis=1), o[:, 6:12].sum(axis=1)])
    # remainder classes (C - 8*CS = 1696): exact on host
    lt = (xn @ mem[N_CORES * CS:].T.astype(np.float64)) / TEMP
    s_tot += np.exp(lt - SHIFT).sum(axis=1)

    # host: exact B-row side terms (target-row routing per sharding hint)
    lse = SHIFT + np.log(s_tot)
    tl = np.einsum("bd,bd->b", xn, mem[tgt], dtype=np.float64) / TEMP
    ce = float(np.mean(lse - tl))
    bbn = bb / np.maximum(np.linalg.norm(bb, axis=1, keepdims=True), EPS)
    g2 = mem[tgt[np.arange(B) // 4]]
    dist = DISTILL_SCALE * float(
        np.sqrt(((bbn.astype(np.float64) - g2) ** 2).sum()))
    return np.asarray(ce + dist, dtype=np.float32)


# revision 25
# speedup vs baseline: 1.0903x; 1.0073x over previous
"""Trainium2 Bass kernel for MemoryBankNet loss (scatter_memory).

Computes, for inputs/backbone_inputs [256,512], targets [256], memory_features
[100000,512]:
    ce   = cross_entropy(l2norm(inputs) @ mem.T / 0.05, targets)
    dist = (0.007/0.3) * ||l2norm(backbone_inputs) - mem[targets[j//4]]||_F
    out  = ce + dist                                    (f32 scalar)

Distribution: classes (mem rows) are sharded 12500/core across 8 NeuronCores
(tensor parallel over the class axis).  Each core computes its partial softmax
denominator with a fixed log-shift; the tiny [256] partials are combined on
host (the "all-reduce" of the softmax normalizer).  The B-row side terms
(target logits for the CE numerator, distill partials) are exact host numpy
over [256,512] gathers -- the same host routing of target rows the hint
describes.

Device numerics: the memory bank and the (pre-normalized, pre-temperature-
scaled) inputs are quantized host-side to fp8e4m3 with power-of-2 scales
(mem*32, l2norm(inp)*20*32), so PSUM = 1024*logit and the activation applies
a CONSTANT scale 2^-10 and bias -104 -- no per-row scale tensors on device.
Validated host-side: rel err ~5.5e-4 vs f64 (tolerance 2e-2).

Device layout per core:
  memT [128, 24*2048+1024] fp8: shard packed as [p][substrip j][k2][i][c]
    with d = k2*256 + i*128 + p, c = class within substrip (512 per substrip,
    tail substrip 256 incl. 44 zero-padded classes -> exp underflows to 0).
  matmul: DoubleRow fp8 (2 fp8/cell, 256-deep contraction per pass):
    stationary = inpT [128, 2, 128] (one b-half, one k2), moving = mem
    substrip [128, 2, 512], psum [128b, 512c], accumulated over 2 k2 passes.
  Per round (2 substrips): 8 MMs fill 4 psum banks; one ACT Exp drains all 4
  into bf16 scratch; DVE row-reduces per half into the sumexp accumulator.
"""

import numpy as np
import ml_dtypes

import concourse.bass as bass
import concourse.tile as tile
from concourse import bacc, mybir
from concourse.bass_utils import run_bass_kernel_spmd

F32 = mybir.dt.float32
FP8 = mybir.dt.float8e4
BF16 = mybir.dt.bfloat16
AF = mybir.ActivationFunctionType
AX = mybir.AxisListType
DR = mybir.MatmulPerfMode.DoubleRow

N_CORES = 8
B, D, C = 256, 512, 100000
CT = 512                     # classes per substrip (one psum bank)
NSUB = 24                    # substrips per core
CS = NSUB * CT               # 12288 device classes per core
CHOST = C - N_CORES * CS     # 1696 remainder classes summed exactly on host
SUB_B = 2 * 2 * CT           # 2048 bytes/partition per substrip
TOT_B = NSUB * SUB_B

TEMP = 0.05
SHIFT = 104.0                # fixed log-shift
ASCALE = 2.0 ** -10          # undo fp8 scales 32*32 = 1024
DISTILL_SCALE = 0.007 / 0.3
EPS = 1e-12

_PROGRAM = None
_last_in_maps = None


def _build_program():
    nc = bacc.Bacc("TRN2", target_bir_lowering=False, debug=False,
                   num_devices=N_CORES)
    memT = nc.dram_tensor("memT", [128, TOT_B], FP8, kind="ExternalInput").ap()
    # stationary: [p][h][k2][i][m] fp8, d = k2*256 + i*128 + p, row = h*128+m
    inpT = nc.dram_tensor("inpT", [128, 2, 2, 2, 128], FP8,
                          kind="ExternalInput").ap()
    # per-(h, group) softmax partials; host does the final 6-column sum
    out = nc.dram_tensor("out", [128, 12], F32, kind="ExternalOutput").ap()

    with tile.TileContext(nc) as tc:
        _body(tc, nc, memT, inpT, out)

    nc.compile()
    return nc


def _body(tc, nc, memT, inpT, out):
    NG = 6                   # ACT groups of 4 substrips = 1MB DMA strips
    NWARM = 10               # junk MMs span a full ~3.4us HAM window at the
                             # cold 427ns issue cadence, so the clock-gate
                             # lifts mid-junk and every real matmul runs warm

    with (
        tc.tile_pool(name="const", bufs=1) as cpool,
        tc.tile_pool(name="exps", bufs=3) as epool,
        tc.tile_pool(name="psum", bufs=2, space="PSUM") as ppool,
    ):
        wt = cpool.tile([128, 2, 2, 2, 128], FP8, tag="wt", name="wt")
        saccw = cpool.tile([128, 2 * NG], F32, tag="saccw", name="saccw")
        nbias = cpool.tile([128, 1], F32, tag="nbias", name="nbias")
        jm = cpool.tile([128, 1024], FP8, tag="jm", name="jm")
        # whole 6.3MB shard is SBUF-resident: strips are persistent tiles,
        # every DMA trigger issues at t=0 with no buffer-reuse waits.
        # All strips ride the sync HWDGE ring alone (uncontended it runs
        # ~340 GB/s; a second queue halves both).  Trigger-issuer choice
        # matters: the 3rd outstanding DMA on a ring BLOCKS its issuing
        # engine, so only the sync engine (otherwise idle) may carry the
        # strip queue; the ACT ring gets just the tiny stationary.
        strips = [cpool.tile([128, 4, 2, 2, CT], FP8, tag=f"mt{s}",
                             name=f"mt{s}") for s in range(NG)]
        nc.scalar.dma_start(wt[:], inpT)
        # strip 0 goes as two 512KB halves so the first matmuls fire ~2us
        # sooner; the rest as full 1MB strips (~400 GB/s on this ring)
        nc.sync.dma_start(
            strips[0][:, 0:2].rearrange("p w k i c -> p (w k i c)"),
            memT[:, 0:2 * SUB_B])
        nc.sync.dma_start(
            strips[0][:, 2:4].rearrange("p w k i c -> p (w k i c)"),
            memT[:, 2 * SUB_B:4 * SUB_B])
        for s in range(1, NG):
            nc.sync.dma_start(
                strips[s][:].rearrange("p w k i c -> p (w k i c)"),
                memT[:, s * 4 * SUB_B:(s + 1) * 4 * SUB_B])
        nc.vector.memset(nbias[:], -SHIFT)
        nc.vector.memset(jm[:], 0.0)

        # warm-up: zero matmuls keep the PE busy from preamble-end until
        # strip 0 lands, so the HAM clock-gate hits 8/8 as real work starts
        ps0 = ppool.tile([128, 4, CT], F32, tag="ps", name="ps")
        jw = jm[:, 0:256].rearrange("p (i m) -> p i m", i=2)
        jr = jm[:].rearrange("p (i c) -> p i c", i=2)
        for _ in range(NWARM):
            nc.tensor.matmul(ps0[:, 0, :], jw, jr, start=True, stop=True,
                             perf_mode=DR, skip_group_check=True)

        # Each group: per half h, 8 matmuls fill 4 psum banks, then a single
        # ACT Exp drains them, accum_out producing the (h, group) softmax
        # partial.  PE fills the other half's banks while ACT drains --
        # psum pool holds 2x4 banks.
        for g in range(NG):
            for h in range(2):
                ps = ps0 if (g == 0 and h == 0) else ppool.tile(
                    [128, 4, CT], F32, tag="ps", name="ps")
                for k2 in range(2):
                    for jj in range(4):
                        nc.tensor.matmul(
                            ps[:, jj, :],
                            wt[:, h, k2],
                            strips[g][:, jj, k2],
                            start=(k2 == 0), stop=(k2 == 1),
                            perf_mode=DR, skip_group_check=True)

                ex = epool.tile([128, 4 * CT], BF16, tag="ex", name="ex")
                col = saccw[:, h * NG + g:h * NG + g + 1]
                # only the final group's rounds (on the critical out-DMA
                # tail) fold the row-sum into the ACT accumulator; all other
                # rounds leave it to the otherwise-idle DVE, saving the
                # 187ns accumulator read on those ACT instructions
                on_act = g == NG - 1
                nc.scalar.activation(
                    ex[:],
                    ps[:].rearrange("p b c -> p (b c)"),
                    AF.Exp, bias=nbias[:], scale=ASCALE,
                    accum_out=col if on_act else None)
                if not on_act:
                    nc.vector.reduce_sum(col, ex[:], axis=AX.X)

        nc.sync.dma_start(out, saccw[:])


def _get_program():
    global _PROGRAM
    if _PROGRAM is None:
        _PROGRAM = _build_program()
    return _PROGRAM


def _quant_fp8(x):
    return np.clip(x, -240.0, 240.0).astype(ml_dtypes.float8_e4m3)


def kernel(backbone_inputs, inputs, targets, memory_features, **_unused):
    x = np.ascontiguousarray(inputs, dtype=np.float32)
    bb = np.ascontiguousarray(backbone_inputs, dtype=np.float32)
    mem = np.ascontiguousarray(memory_features, dtype=np.float32)
    tgt = np.asarray(targets).astype(np.int64)

    xn = x / np.maximum(np.linalg.norm(x, axis=1, keepdims=True), EPS)

    # stationary: value[p,h,k2,i,m] = (xn*640)[h*128+m, k2*256+i*128+p]
    qi = _quant_fp8(xn * (32.0 / TEMP))
    spt8 = np.ascontiguousarray(
        qi.reshape(2, 128, 2, 2, 128).transpose(4, 0, 2, 3, 1))

    qm = _quant_fp8(mem * 32.0)

    nc = _get_program()
    in_maps = []
    for c in range(N_CORES):
        sh = qm[c * CS:(c + 1) * CS].reshape(NSUB, CT, 2, 2, 128)
        shard = np.ascontiguousarray(sh.transpose(4, 0, 2, 3, 1)).reshape(128, -1)
        in_maps.append({"memT": shard, "inpT": spt8})
    global _last_in_maps
    _last_in_maps = in_maps
    results = run_bass_kernel_spmd(nc, in_maps, core_ids=list(range(N_CORES)))

    s_tot = np.zeros(B, dtype=np.float64)
    for r in results.results:
        o = r["out"].astype(np.float64)        # [128, 2*6] (h, group) partials
        s_tot += np.concatenate([o[:, 0:6].sum(axis=1), o[:, 6:12].sum(axis=1)])
    # remainder classes (C - 8*CS = 1696): exact on host
    lt = (xn @ mem[N_CORES * CS:].T.astype(np.float64)) / TEMP
    s_tot += np.exp(lt - SHIFT).sum(axis=1)

    # host: exact B-row side terms (target-row routing per sharding hint)
    lse = SHIFT + np.log(s_tot)
    tl = np.einsum("bd,bd->b", xn, mem[tgt], dtype=np.float64) / TEMP
    ce = float(np.mean(lse - tl))
    bbn = bb / np.maximum(np.linalg.norm(bb, axis=1, keepdims=True), EPS)
    g2 = mem[tgt[np.arange(B) // 4]]
    dist = DISTILL_SCALE * float(
        np.sqrt(((bbn.astype(np.float64) - g2) ** 2).sum()))
    return np.asarray(ce + dist, dtype=np.float32)
